# revision 55
# baseline (speedup 1.0000x reference)
"""Multi-head self-attention TRN2 kernel (16 heads, D=1024, x:[2,2048,1024]).

Sharding: 8 cores = 2 (batch) x 4 (head groups of 4 heads). Host sums the
4 bf16 partials per batch (tensor-parallel all-reduce) and adds bo.

Per-core pipeline:
  QKV proj: error-compensated fp8e4m3 DoubleRow (half-rate PE):
        x@w ~= x8@w8 + xr8@w8 + x8s@wr8s  with host-prepared x8=e4m3(x),
        xr8=e4m3(x-x8), x8s=e4m3(x/32), wr8s=e4m3((w-w8)*32); kT/qT
        head-dim-major bf16 [256, 2048], v token-major [2048, 4, 65] bf16
        with a ones column (softmax sums fall out of the AV matmul).
  scores:  kT strips (K=64) x qT -> [128 keys, 512 q] f32 PSUM, bf16 ops.
  ex:      exp(s/8): ACT Exp (bf16 out) for 3/4 of (head, key-group)
           slices; DVE Schraudolph (affine+uint16 trunc = bf16 exp bits,
           mean-centered) for the rest -- engine-split is per whole
           key-group so softmax normalization cancels engine bias.
  o^T:     AV in q-major orientation: [128 q, 65] = ex_chunk^T @ v_chunk,
           4 q-subblocks packed per PSUM bank (one start=True zeroes the
           2KB bank; the rest accumulate onto pending-zero bytes).
  norm:    DVE reciprocal_approx_fast on the sums column + broadcast mult.
  oT:      XBAR DMA transpose [128 q, 128 dh-pair] -> oT_sb bf16.
  out:     oT^T @ wo (bf16) per 128-token chunk -> DVE bf16 copy -> DMA.
Projections for head-pair 1 / later q-blocks and the output projection
are streamed into the attention stages to hide the input DMA and keep
all engines busy.
"""

import os
import sys
from contextlib import ExitStack

import numpy as np

for _p in ("/opt/trn_rl_repo", os.path.expanduser("~/.axon_site/_ro/trn_rl_repo")):
    if os.path.isdir(_p) and _p not in sys.path:
        sys.path.insert(0, _p)

import ml_dtypes  # noqa: E402

import concourse.bass as bass  # noqa: E402
import concourse.mybir as mybir  # noqa: E402
import concourse.tile as tile  # noqa: E402
from concourse import bacc  # noqa: E402
from concourse.bass_utils import run_bass_kernel_spmd  # noqa: E402

f32 = mybir.dt.float32
bf16 = mybir.dt.bfloat16
u16 = mybir.dt.uint16
P = 128

# Schraudolph exp-as-bf16-bits: bits = A*s_raw + B, trunc to uint16.
# A = 128*log2(e)*0.125 (the 1/8 softmax scale folded in); B centers the
# piecewise-linear ripple (minimax) and compensates truncation.
SCH_A = 128.0 * float(np.log2(np.e)) * 0.125
SCH_B = 128.0 * 127.0 - 6.86


def build_core_program(D=1024, TOK=2048, NH=4, num_devices=8):
    """One core's program: 4 heads of one batch of the attention layer."""
    DH = 64
    KD = D // P          # hidden-dim 128-chunks (8)
    NQ = TOK // 512      # 512-wide q blocks (4)
    NT = TOK // P        # 128-wide token chunks (16)
    DC = NH * DH         # per-core head dims (256)
    MQ = DC // P         # 128-row chunks of qT/kT/oT (2)
    HPC = P // DH        # heads per 128-row chunk (2)
    OW = 512             # output column chunk width
    NO = D // OW         # output column chunks (2)
    G = NT // 2          # key-pair groups per stage (8)

    nc = bacc.Bacc("TRN2", target_bir_lowering=False, debug=False,
                   num_devices=num_devices)

    KC = KD // 2         # 256-row DoubleRow chunks (4)
    fp8 = mybir.dt.float8e4
    x8_d = nc.declare_dram_parameter("x8", [P, KC, 2, TOK], fp8, isOutput=False)
    xr_d = nc.declare_dram_parameter("xr", [P, KC, 2, TOK], fp8, isOutput=False)
    xs_d = nc.declare_dram_parameter("xs", [P, KC, 2, TOK], fp8, isOutput=False)
    w8_d = {}
    wr_d = {}
    for nm in ("q", "k", "v"):
        w8_d[nm] = nc.declare_dram_parameter(
            f"w8{nm}", [P, KC, 2, DC], fp8, isOutput=False)
        wr_d[nm] = nc.declare_dram_parameter(
            f"wr{nm}", [P, KC, 2, DC], fp8, isOutput=False)
    wo_d = nc.declare_dram_parameter("wo", [P, MQ, D], bf16, isOutput=False)
    bq_d = nc.declare_dram_parameter("bq", [P, MQ], f32, isOutput=False)
    bk_d = nc.declare_dram_parameter("bk", [P, MQ], f32, isOutput=False)
    bv_d = nc.declare_dram_parameter("bv", [P, DC], f32, isOutput=False)
    out_d = nc.declare_dram_parameter("out", [TOK, D], bf16, isOutput=True)

    with tile.TileContext(nc) as tc, ExitStack() as ctx:
        persist = ctx.enter_context(tc.tile_pool(name="persist", bufs=1))
        work = ctx.enter_context(tc.tile_pool(name="work", bufs=3))
        psp = ctx.enter_context(tc.tile_pool(name="psp", bufs=2, space="PSUM"))

        # ---- phase A: loads ------------------------------------------
        w8_sb = {nm: persist.tile([P, KC, 2, DC], fp8, name=f"w8{nm}")
                 for nm in ("q", "k", "v")}
        wr_sb = {nm: persist.tile([P, KC, 2, DC], fp8, name=f"wr{nm}")
                 for nm in ("q", "k", "v")}
        wo_sb = persist.tile([P, MQ, D], bf16)
        bq_sb = persist.tile([P, MQ], f32)
        bk_sb = persist.tile([P, MQ], f32)
        bv_sb = persist.tile([P, DC], f32)
        x8_sb = persist.tile([P, KC, 2, TOK], fp8)
        xr_sb = persist.tile([P, KC, 2, TOK], fp8)
        xs_sb = persist.tile([P, KC, 2, TOK], fp8)

        # startup-critical transfers first (kT0/qT0 2-set inputs), then the
        # stream in stage-(0,*) consumption order
        nc.gpsimd.dma_start(w8_sb["k"][:], w8_d["k"][:])
        nc.sync.dma_start(x8_sb[:, :, :, 0:512], x8_d[:, :, :, 0:512])
        nc.gpsimd.dma_start(w8_sb["q"][:], w8_d["q"][:])
        nc.sync.dma_start(xr_sb[:, :, :, 0:512], xr_d[:, :, :, 0:512])
        nc.gpsimd.dma_start(bk_sb[:], bk_d[:])
        nc.gpsimd.dma_start(bq_sb[:], bq_d[:])
        nc.gpsimd.dma_start(wr_sb["k"][:], wr_d["k"][:])
        nc.sync.dma_start(xs_sb[:, :, :, 0:512], xs_d[:, :, :, 0:512])
        nc.gpsimd.dma_start(wr_sb["q"][:], wr_d["q"][:])
        nc.gpsimd.dma_start(w8_sb["v"][:], w8_d["v"][:])
        nc.gpsimd.dma_start(wr_sb["v"][:], wr_d["v"][:])
        nc.gpsimd.dma_start(bv_sb[:], bv_d[:])
        for n in range(1, NQ):
            ns = slice(n * 512, (n + 1) * 512)
            for t_sb, t_d in ((x8_sb, x8_d), (xr_sb, xr_d), (xs_sb, xs_d)):
                nc.sync.dma_start(t_sb[:, :, :, ns], t_d[:, :, :, ns])
        nc.gpsimd.dma_start(wo_sb[:], wo_d[:])

        v_sb = persist.tile([P, NT, NH, DH + 1], bf16)
        nc.vector.memset(v_sb[:, :, :, DH:DH + 1], 1.0)

        qT_sb = persist.tile([P, MQ, TOK], bf16)
        kT_sb = persist.tile([P, MQ, TOK], bf16)
        oT_sb = persist.tile([P, MQ, TOK], bf16)

        # ---- phase B: kT/qT0 for head-pair 0 only; the rest of the
        # projections are interleaved into the first two stages --------
        DR = mybir.MatmulPerfMode.DoubleRow
        PROJ_SETS = (("8", x8_sb), ("8", xr_sb), ("r", xs_sb))

        def proj_block(nm, b_sb, t_sb, m, n, nsets=3):
            # t = (x8@w8 + xr8@w8 + x8s@wr8s) + b, fp8e4 DoubleRow
            ps = psp.tile([P, 512], f32, tag="acc", name="ps")
            ns = slice(n * 512, (n + 1) * 512)
            wsb = {"8": w8_sb[nm], "r": wr_sb[nm]}
            for si, (wv_, xv) in enumerate(PROJ_SETS[:nsets]):
                for c in range(KC):
                    nc.tensor.matmul(
                        ps[:], wsb[wv_][:, c, :, m * P:(m + 1) * P],
                        xv[:, c, :, ns], perf_mode=DR,
                        start=(si == 0 and c == 0),
                        stop=(si == nsets - 1 and c == KC - 1))
            nc.vector.tensor_tensor(
                t_sb[:, m, ns], ps[:],
                b_sb[:, m:m + 1].to_broadcast([P, 512]),
                mybir.AluOpType.add)

        # first two blocks skip the w-residual set: 3 fewer DMA transfers
        # on the startup critical path, ~0.03% extra noise on those columns
        proj_block("k", bk_sb, kT_sb, 0, 0)
        proj_block("q", bq_sb, qT_sb, 0, 0)

        def emit_v_chunk(t, vp):
            ps = psp.tile([P, P], f32, tag="acc", name="vps")
            tsl = slice(t * P, (t + 1) * P)
            vsl = slice(vp * P, (vp + 1) * P)
            wsb = {"8": w8_sb["v"], "r": wr_sb["v"]}
            for si, (wv_, xv) in enumerate(PROJ_SETS):
                for c in range(KC):
                    nc.tensor.matmul(
                        ps[:], xv[:, c, :, tsl], wsb[wv_][:, c, :, vsl],
                        perf_mode=DR,
                        start=(si == 0 and c == 0),
                        stop=(si == 2 and c == KC - 1))
            nc.vector.tensor_tensor(
                v_sb[:, t, 2 * vp:2 * vp + 2, 0:DH],
                ps.rearrange("p (h d) -> p h d", h=HPC),
                bv_sb[:, vsl].rearrange("p (h d) -> p h d", h=HPC),
                mybir.AluOpType.add)

        # ---- phase C: attention + output projection ------------------
        def emit_oproj_tok(tok, fin=False):
            ts = slice(tok * P, (tok + 1) * P)
            for nn in range(NO):
                ns = slice(nn * OW, (nn + 1) * OW)
                op = psp.tile([P, OW], f32, tag="acc", name="op")
                for m in range(MQ):
                    nc.tensor.matmul(
                        op[:], oT_sb[:, m, ts], wo_sb[:, m, ns],
                        start=(m == 0), stop=(m == MQ - 1))
                ou = work.tile([P, OW], bf16, tag="ou", bufs=4, name="ou")
                nc.vector.tensor_copy(ou[:], op[:])
                (nc.sync if fin else nc.gpsimd).dma_start(
                    out_d[ts, ns], ou[:])

        def emit_oproj(n):
            for t in range(4):
                emit_oproj_tok(n * 4 + t)

        for n in range(NQ):
            qs = slice(n * 512, (n + 1) * 512)
            for hp in range(MQ):
                pair = [hp * HPC, hp * HPC + 1]
                avs = {}
                for h in pair:
                    avs[h] = psp.tile([P, 4, DH + 1], f32, tag="av",
                                      padded_shape=[P, 4, P], name=f"av{h}")

                def emit_av(g, exs, which):
                    for h in which:
                        ex = exs[h]
                        for j in range(2):
                            for q4 in range(4):
                                nc.tensor.matmul(
                                    avs[h][:, q4, :],
                                    ex[:, j, q4 * P:(q4 + 1) * P],
                                    v_sb[:, 2 * g + j, h, :],
                                    start=(g == 0 and j == 0 and q4 == 0),
                                    stop=(g == G - 1 and j == 1),
                                    skip_group_check=True)

                exs_hist = {}
                for g in range(G):
                    if n == 0 and g % 2 == 0 and g > 0:
                        # stream the kT block feeding this key-group pair
                        proj_block("k", bk_sb, kT_sb, hp, g // 2)
                    if n == 0 and hp == 0 and g == 1:
                        proj_block("k", bk_sb, kT_sb, 1, 0)

                    cur_exs = {}
                    for i, h in enumerate(pair):
                        sc = psp.tile([P, 2, 512], f32, tag="sc",
                                      name=f"sc{h}")
                        hm, hr = h // HPC, (h % HPC) * DH
                        for j in range(2):
                            kk = g * 2 + j
                            nc.tensor.matmul(
                                sc[:, j, :],
                                kT_sb[hr:hr + DH, hm, kk * P:(kk + 1) * P],
                                qT_sb[hr:hr + DH, hm, qs],
                                start=True, stop=True)
                        # exp: DVE Schraudolph for ~5/16 of (head, key-group)
                        # slices; ACT exact exp otherwise
                        if DVE_EXP and ((i == 1 and g % 2 == 0) or (i == 0 and g == 3)):
                            ex = work.tile([P, 2, 512], u16, bufs=4,
                                           tag=f"ex{i}", name="exu")
                            nc.vector.tensor_scalar(
                                ex[:], sc[:], SCH_A, SCH_B,
                                mybir.AluOpType.mult, mybir.AluOpType.add)
                            cur_exs[h] = ex.bitcast(bf16)
                        else:
                            ex = work.tile([P, 2, 512], bf16, bufs=4,
                                           tag=f"ex{i}", name="exb")
                            nc.scalar.activation(
                                ex[:], sc[:],
                                mybir.ActivationFunctionType.Exp, scale=0.125)
                            cur_exs[h] = ex
                        # AV lagged two groups behind scores/exp to absorb
                        # exp latency; halved between the heads' emissions
                        if g >= 2:
                            emit_av(g - 2, exs_hist[g - 2], [pair[i]])
                    if n == 0:
                        # v projections streamed just ahead of their AV use
                        emit_v_chunk(2 * g, hp)
                        emit_v_chunk(2 * g + 1, hp)
                        if hp == 0 and g == G - 1:
                            proj_block("q", bq_sb, qT_sb, 1, 0)
                    elif hp == 1 and g < 4:
                        # previous block's output projection, one token
                        # chunk per key-group to spread PSUM slot reuse
                        emit_oproj_tok((n - 1) * 4 + g, fin=(n == NQ - 1))
                    exs_hist[g] = cur_exs
                emit_av(G - 2, exs_hist[G - 2], pair)
                emit_av(G - 1, exs_hist[G - 1], pair)
                # normalize + transpose into oT (+ last-block oproj tail)
                last = (n == NQ - 1 and hp == MQ - 1)
                onorm = work.tile([P, 4, P], bf16, tag="onorm", bufs=3,
                                  name="onorm")
                for i, h in enumerate(pair):
                    rcp = work.tile([P, 4, 1], f32, tag="rcp", bufs=4,
                                    name="rcp")
                    nc.vector.reciprocal_approx_fast(
                        rcp[:], avs[h][:, :, DH:DH + 1])
                    nc.vector.tensor_tensor(
                        onorm[:, :, i * DH:(i + 1) * DH],
                        avs[h][:, :, 0:DH],
                        rcp.to_broadcast([P, 4, DH]),
                        mybir.AluOpType.mult)
                if n + 1 < NQ:
                    proj_block("q", bq_sb, qT_sb, hp, n + 1)
                for q4 in range(4):
                    nc.sync.dma_start_transpose(
                        oT_sb[:, hp, n * 512 + q4 * P: n * 512 + (q4 + 1) * P],
                        onorm[:, q4, :])
                    if last:
                        emit_oproj_tok(n * 4 + q4, fin=True)
    return nc


_CACHE = {}
LAST_RESULTS = None


def _get_compiled():
    if "nc" not in _CACHE:
        nc = build_core_program()
        nc.compile()
        _CACHE["nc"] = nc
    return _CACHE["nc"]


def kernel(x, wq, bq, wk, bk, wv, bv, wo, bo):
    global LAST_RESULTS
    x = np.asarray(x, np.float32)
    wq, bq = np.asarray(wq, np.float32), np.asarray(bq, np.float32)
    wk, bk = np.asarray(wk, np.float32), np.asarray(bk, np.float32)
    wv, bv = np.asarray(wv, np.float32), np.asarray(bv, np.float32)
    wo, bo = np.asarray(wo, np.float32), np.asarray(bo, np.float32)
    B, TOK, D = x.shape          # (2, 2048, 1024)
    NH, DH = 4, 64               # heads per core, head dim
    DC = NH * DH                 # 256
    MQ = DC // P                 # 2
    KD = D // P                  # 8
    KC = KD // 2                 # 4
    BF = ml_dtypes.bfloat16
    E4 = ml_dtypes.float8_e4m3

    nc = _get_compiled()

    def chunk_rows(a, nchunk):
        # [R, C] -> [P, nchunk, C] with [p, c, :] = a[c*P + p, :]
        R, C = a.shape
        return np.ascontiguousarray(
            a.reshape(nchunk, P, C).transpose(1, 0, 2))

    def dr_chunks(a):
        # [R, C] -> [P, KC, 2, C] DoubleRow layout (row = c*256 + j*128 + p)
        c = chunk_rows(a, KD)  # [P, 8, C]
        return np.ascontiguousarray(
            c.reshape(P, KC, 2, a.shape[1]))

    def fp8_sets(a):
        # returns (a8, ar8, a8s): value + residual + 2^-5-scaled copies
        a8 = a.astype(E4)
        ar = (a - a8.astype(np.float32)).astype(E4)
        as_ = (a * 2.0 ** -5).astype(E4)
        return a8, ar, as_

    in_maps = []
    x8_b, xr_b, xs_b = [], [], []
    for b in range(B):
        x8, xr, xs = fp8_sets(x[b].T)  # [D, TOK]
        x8_b.append(dr_chunks(x8))
        xr_b.append(dr_chunks(xr))
        xs_b.append(dr_chunks(xs))
    for c in range(8):
        b, hg = c // 4, c % 4
        sl = slice(hg * DC, (hg + 1) * DC)
        m = {
            "x8": x8_b[b], "xr": xr_b[b], "xs": xs_b[b],
            "wo": chunk_rows(wo[sl, :], MQ).astype(BF),
            "bq": np.ascontiguousarray(bq[sl].reshape(MQ, P).T),
            "bk": np.ascontiguousarray(bk[sl].reshape(MQ, P).T),
            "bv": np.ascontiguousarray(np.tile(bv[None, sl], (P, 1))),
        }
        for nm, w in (("q", wq), ("k", wk), ("v", wv)):
            w8 = w[:, sl].astype(E4)
            wr = ((w[:, sl] - w8.astype(np.float32)) * 2.0 ** 5).astype(E4)
            m[f"w8{nm}"] = dr_chunks(w8)
            m[f"wr{nm}"] = dr_chunks(wr)
        in_maps.append(m)

    trace = os.environ.get("KERNEL_TRACE", "0") == "1"
    res = run_bass_kernel_spmd(nc, in_maps, core_ids=list(range(8)),
                               trace=trace)
    LAST_RESULTS = res
    outs = [res.results[c]["out"].astype(np.float32) for c in range(8)]
    y = np.stack([sum(outs[0:4]) + bo, sum(outs[4:8]) + bo], axis=0)
    return np.ascontiguousarray(y, dtype=np.float32)


# revision 58
# speedup vs baseline: 1.0004x; 1.0004x over previous
"""Multi-head self-attention TRN2 kernel (16 heads, D=1024, x:[2,2048,1024]).

Sharding: 8 cores = 2 (batch) x 4 (head groups of 4 heads). Host sums the
4 bf16 partials per batch (tensor-parallel all-reduce) and adds bo.

Per-core pipeline:
  QKV proj: error-compensated fp8e4m3 DoubleRow (half-rate PE):
        x@w ~= x8@w8 + xr8@w8 + x8s@wr8s  with host-prepared x8=e4m3(x),
        xr8=e4m3(x-x8), x8s=e4m3(x/32), wr8s=e4m3((w-w8)*32); kT/qT
        head-dim-major bf16 [256, 2048], v token-major [2048, 4, 65] bf16
        with a ones column (softmax sums fall out of the AV matmul).
  scores:  kT strips (K=64) x qT -> [128 keys, 512 q] f32 PSUM, bf16 ops.
  ex:      exp(s/8): ACT Exp (bf16 out) for 3/4 of (head, key-group)
           slices; DVE Schraudolph (affine+uint16 trunc = bf16 exp bits,
           mean-centered) for the rest -- engine-split is per whole
           key-group so softmax normalization cancels engine bias.
  o^T:     AV in q-major orientation: [128 q, 65] = ex_chunk^T @ v_chunk,
           4 q-subblocks packed per PSUM bank (one start=True zeroes the
           2KB bank; the rest accumulate onto pending-zero bytes).
  norm:    DVE reciprocal_approx_fast on the sums column + broadcast mult.
  oT:      XBAR DMA transpose [128 q, 128 dh-pair] -> oT_sb bf16.
  out:     oT^T @ wo (bf16) per 128-token chunk -> DVE bf16 copy -> DMA.
Projections for head-pair 1 / later q-blocks and the output projection
are streamed into the attention stages to hide the input DMA and keep
all engines busy.
"""

import os
import sys
from contextlib import ExitStack

import numpy as np

for _p in ("/opt/trn_rl_repo", os.path.expanduser("~/.axon_site/_ro/trn_rl_repo")):
    if os.path.isdir(_p) and _p not in sys.path:
        sys.path.insert(0, _p)

import ml_dtypes  # noqa: E402

import concourse.bass as bass  # noqa: E402
import concourse.mybir as mybir  # noqa: E402
import concourse.tile as tile  # noqa: E402
from concourse import bacc  # noqa: E402
from concourse.bass_utils import run_bass_kernel_spmd  # noqa: E402

f32 = mybir.dt.float32
bf16 = mybir.dt.bfloat16
u16 = mybir.dt.uint16
P = 128

# Schraudolph exp-as-bf16-bits: bits = A*s_raw + B, trunc to uint16.
# A = 128*log2(e)*0.125 (the 1/8 softmax scale folded in); B centers the
# piecewise-linear ripple (minimax) and compensates truncation.
SCH_A = 128.0 * float(np.log2(np.e)) * 0.125
SCH_B = 128.0 * 127.0 - 6.86


def build_core_program(D=1024, TOK=2048, NH=4, num_devices=8):
    """One core's program: 4 heads of one batch of the attention layer."""
    DH = 64
    KD = D // P          # hidden-dim 128-chunks (8)
    NQ = TOK // 512      # 512-wide q blocks (4)
    NT = TOK // P        # 128-wide token chunks (16)
    DC = NH * DH         # per-core head dims (256)
    MQ = DC // P         # 128-row chunks of qT/kT/oT (2)
    HPC = P // DH        # heads per 128-row chunk (2)
    OW = 512             # output column chunk width
    NO = D // OW         # output column chunks (2)
    G = NT // 2          # key-pair groups per stage (8)

    nc = bacc.Bacc("TRN2", target_bir_lowering=False, debug=False,
                   num_devices=num_devices)

    KC = KD // 2         # 256-row DoubleRow chunks (4)
    fp8 = mybir.dt.float8e4
    x8_d = nc.declare_dram_parameter("x8", [P, KC, 2, TOK], fp8, isOutput=False)
    xr_d = nc.declare_dram_parameter("xr", [P, KC, 2, TOK], fp8, isOutput=False)
    xs_d = nc.declare_dram_parameter("xs", [P, KC, 2, TOK], fp8, isOutput=False)
    w8_d = {}
    wr_d = {}
    for nm in ("q", "k", "v"):
        w8_d[nm] = nc.declare_dram_parameter(
            f"w8{nm}", [P, KC, 2, DC], fp8, isOutput=False)
        wr_d[nm] = nc.declare_dram_parameter(
            f"wr{nm}", [P, KC, 2, DC], fp8, isOutput=False)
    wo_d = nc.declare_dram_parameter("wo", [P, MQ, D], bf16, isOutput=False)
    bq_d = nc.declare_dram_parameter("bq", [P, MQ], f32, isOutput=False)
    bk_d = nc.declare_dram_parameter("bk", [P, MQ], f32, isOutput=False)
    bv_d = nc.declare_dram_parameter("bv", [P, DC], f32, isOutput=False)
    out_d = nc.declare_dram_parameter("out", [TOK, D], bf16, isOutput=True)

    with tile.TileContext(nc) as tc, ExitStack() as ctx:
        persist = ctx.enter_context(tc.tile_pool(name="persist", bufs=1))
        work = ctx.enter_context(tc.tile_pool(name="work", bufs=3))
        psp = ctx.enter_context(tc.tile_pool(name="psp", bufs=2, space="PSUM"))

        # ---- phase A: loads ------------------------------------------
        w8_sb = {nm: persist.tile([P, KC, 2, DC], fp8, name=f"w8{nm}")
                 for nm in ("q", "k", "v")}
        wr_sb = {nm: persist.tile([P, KC, 2, DC], fp8, name=f"wr{nm}")
                 for nm in ("q", "k", "v")}
        wo_sb = persist.tile([P, MQ, D], bf16)
        bq_sb = persist.tile([P, MQ], f32)
        bk_sb = persist.tile([P, MQ], f32)
        bv_sb = persist.tile([P, DC], f32)
        x8_sb = persist.tile([P, KC, 2, TOK], fp8)
        xr_sb = persist.tile([P, KC, 2, TOK], fp8)
        xs_sb = persist.tile([P, KC, 2, TOK], fp8)

        # startup-critical transfers first (kT0/qT0 2-set inputs), then the
        # stream in stage-(0,*) consumption order
        nc.gpsimd.dma_start(w8_sb["k"][:], w8_d["k"][:])
        nc.sync.dma_start(x8_sb[:, :, :, 0:512], x8_d[:, :, :, 0:512])
        nc.gpsimd.dma_start(w8_sb["q"][:], w8_d["q"][:])
        nc.sync.dma_start(xr_sb[:, :, :, 0:512], xr_d[:, :, :, 0:512])
        nc.gpsimd.dma_start(bk_sb[:], bk_d[:])
        nc.gpsimd.dma_start(bq_sb[:], bq_d[:])
        nc.gpsimd.dma_start(wr_sb["k"][:], wr_d["k"][:])
        nc.sync.dma_start(xs_sb[:, :, :, 0:512], xs_d[:, :, :, 0:512])
        nc.gpsimd.dma_start(wr_sb["q"][:], wr_d["q"][:])
        nc.gpsimd.dma_start(w8_sb["v"][:], w8_d["v"][:])
        nc.gpsimd.dma_start(wr_sb["v"][:], wr_d["v"][:])
        nc.gpsimd.dma_start(bv_sb[:], bv_d[:])
        for n in range(1, NQ):
            ns = slice(n * 512, (n + 1) * 512)
            for t_sb, t_d in ((x8_sb, x8_d), (xr_sb, xr_d), (xs_sb, xs_d)):
                nc.sync.dma_start(t_sb[:, :, :, ns], t_d[:, :, :, ns])
        nc.gpsimd.dma_start(wo_sb[:], wo_d[:])

        v_sb = persist.tile([P, NT, NH, DH + 1], bf16)
        nc.vector.memset(v_sb[:, :, :, DH:DH + 1], 1.0)

        qT_sb = persist.tile([P, MQ, TOK], bf16)
        kT_sb = persist.tile([P, MQ, TOK], bf16)
        oT_sb = persist.tile([P, MQ, TOK], bf16)

        # ---- phase B: kT/qT0 for head-pair 0 only; the rest of the
        # projections are interleaved into the first two stages --------
        DR = mybir.MatmulPerfMode.DoubleRow
        PROJ_SETS = (("8", x8_sb), ("8", xr_sb), ("r", xs_sb))

        def proj_block(nm, b_sb, t_sb, m, n, nsets=3):
            # t = (x8@w8 + xr8@w8 + x8s@wr8s) + b, fp8e4 DoubleRow
            ps = psp.tile([P, 512], f32, tag="acc", name="ps")
            ns = slice(n * 512, (n + 1) * 512)
            wsb = {"8": w8_sb[nm], "r": wr_sb[nm]}
            for si, (wv_, xv) in enumerate(PROJ_SETS[:nsets]):
                for c in range(KC):
                    nc.tensor.matmul(
                        ps[:], wsb[wv_][:, c, :, m * P:(m + 1) * P],
                        xv[:, c, :, ns], perf_mode=DR,
                        start=(si == 0 and c == 0),
                        stop=(si == nsets - 1 and c == KC - 1))
            nc.vector.tensor_tensor(
                t_sb[:, m, ns], ps[:],
                b_sb[:, m:m + 1].to_broadcast([P, 512]),
                mybir.AluOpType.add)

        # first two blocks skip the w-residual set: 3 fewer DMA transfers
        # on the startup critical path, ~0.03% extra noise on those columns
        proj_block("k", bk_sb, kT_sb, 0, 0)
        proj_block("q", bq_sb, qT_sb, 0, 0)

        def emit_v_chunk(t, vp):
            ps = psp.tile([P, P], f32, tag="acc", name="vps")
            tsl = slice(t * P, (t + 1) * P)
            vsl = slice(vp * P, (vp + 1) * P)
            wsb = {"8": w8_sb["v"], "r": wr_sb["v"]}
            for si, (wv_, xv) in enumerate(PROJ_SETS):
                for c in range(KC):
                    nc.tensor.matmul(
                        ps[:], xv[:, c, :, tsl], wsb[wv_][:, c, :, vsl],
                        perf_mode=DR,
                        start=(si == 0 and c == 0),
                        stop=(si == 2 and c == KC - 1))
            nc.vector.tensor_tensor(
                v_sb[:, t, 2 * vp:2 * vp + 2, 0:DH],
                ps.rearrange("p (h d) -> p h d", h=HPC),
                bv_sb[:, vsl].rearrange("p (h d) -> p h d", h=HPC),
                mybir.AluOpType.add)

        # ---- phase C: attention + output projection ------------------
        def emit_oproj_tok(tok, fin=False):
            ts = slice(tok * P, (tok + 1) * P)
            for nn in range(NO):
                ns = slice(nn * OW, (nn + 1) * OW)
                op = psp.tile([P, OW], f32, tag="acc", name="op")
                for m in range(MQ):
                    nc.tensor.matmul(
                        op[:], oT_sb[:, m, ts], wo_sb[:, m, ns],
                        start=(m == 0), stop=(m == MQ - 1))
                ou = work.tile([P, OW], bf16, tag="ou", bufs=4, name="ou")
                nc.vector.tensor_copy(ou[:], op[:])
                (nc.sync if fin else nc.gpsimd).dma_start(
                    out_d[ts, ns], ou[:])

        def emit_oproj(n):
            for t in range(4):
                emit_oproj_tok(n * 4 + t)

        for n in range(NQ):
            qs = slice(n * 512, (n + 1) * 512)
            for hp in range(MQ):
                pair = [hp * HPC, hp * HPC + 1]
                avs = {}
                for h in pair:
                    avs[h] = psp.tile([P, 4, DH + 1], f32, tag="av",
                                      padded_shape=[P, 4, P], name=f"av{h}")

                def emit_av(g, exs, which):
                    for h in which:
                        ex = exs[h]
                        for j in range(2):
                            for q4 in range(4):
                                nc.tensor.matmul(
                                    avs[h][:, q4, :],
                                    ex[:, j, q4 * P:(q4 + 1) * P],
                                    v_sb[:, 2 * g + j, h, :],
                                    start=(g == 0 and j == 0 and q4 == 0),
                                    stop=(g == G - 1 and j == 1),
                                    skip_group_check=True)

                exs_hist = {}
                for g in range(G):
                    if n == 0 and g % 2 == 0 and g > 0:
                        # stream the kT block feeding this key-group pair
                        proj_block("k", bk_sb, kT_sb, hp, g // 2)
                    if n == 0 and hp == 0 and g == 1:
                        proj_block("k", bk_sb, kT_sb, 1, 0)

                    cur_exs = {}
                    for i, h in enumerate(pair):
                        sc = psp.tile([P, 2, 512], f32, tag="sc",
                                      name=f"sc{h}")
                        hm, hr = h // HPC, (h % HPC) * DH
                        for j in range(2):
                            kk = g * 2 + j
                            nc.tensor.matmul(
                                sc[:, j, :],
                                kT_sb[hr:hr + DH, hm, kk * P:(kk + 1) * P],
                                qT_sb[hr:hr + DH, hm, qs],
                                start=True, stop=True)
                        # exp: DVE Schraudolph for ~5/16 of (head, key-group)
                        # slices; ACT exact exp otherwise
                        if DVE_EXP and ((i == 1 and g % 2 == 0) or (i == 0 and g == 3)):
                            ex = work.tile([P, 2, 512], u16, bufs=6,
                                           tag=f"ex{i}", name="exu")
                            nc.vector.tensor_scalar(
                                ex[:], sc[:], SCH_A, SCH_B,
                                mybir.AluOpType.mult, mybir.AluOpType.add)
                            cur_exs[h] = ex.bitcast(bf16)
                        else:
                            ex = work.tile([P, 2, 512], bf16, bufs=6,
                                           tag=f"ex{i}", name="exb")
                            nc.scalar.activation(
                                ex[:], sc[:],
                                mybir.ActivationFunctionType.Exp, scale=0.125)
                            cur_exs[h] = ex
                        # AV lagged two groups behind scores/exp to absorb
                        # exp latency; halved between the heads' emissions
                        if g >= 2:
                            emit_av(g - 2, exs_hist[g - 2], [pair[i]])
                    if n == 0:
                        # v projections streamed just ahead of their AV use
                        emit_v_chunk(2 * g, hp)
                        emit_v_chunk(2 * g + 1, hp)
                        if hp == 0 and g == G - 1:
                            proj_block("q", bq_sb, qT_sb, 1, 0)
                    elif hp == 1 and g < 4:
                        # previous block's output projection, one token
                        # chunk per key-group to spread PSUM slot reuse
                        emit_oproj_tok((n - 1) * 4 + g, fin=(n == NQ - 1))
                    exs_hist[g] = cur_exs
                emit_av(G - 2, exs_hist[G - 2], pair)
                emit_av(G - 1, exs_hist[G - 1], pair)
                # normalize + transpose into oT (+ last-block oproj tail)
                last = (n == NQ - 1 and hp == MQ - 1)
                onorm = work.tile([P, 4, P], bf16, tag="onorm", bufs=4,
                                  name="onorm")
                for i, h in enumerate(pair):
                    rcp = work.tile([P, 4, 1], f32, tag="rcp", bufs=4,
                                    name="rcp")
                    nc.vector.reciprocal_approx_fast(
                        rcp[:], avs[h][:, :, DH:DH + 1])
                    nc.vector.tensor_tensor(
                        onorm[:, :, i * DH:(i + 1) * DH],
                        avs[h][:, :, 0:DH],
                        rcp.to_broadcast([P, 4, DH]),
                        mybir.AluOpType.mult)
                if n + 1 < NQ:
                    proj_block("q", bq_sb, qT_sb, hp, n + 1)
                for q4 in range(4):
                    nc.sync.dma_start_transpose(
                        oT_sb[:, hp, n * 512 + q4 * P: n * 512 + (q4 + 1) * P],
                        onorm[:, q4, :])
                    if last:
                        emit_oproj_tok(n * 4 + q4, fin=True)
    return nc


_CACHE = {}
LAST_RESULTS = None


def _get_compiled():
    if "nc" not in _CACHE:
        nc = build_core_program()
        nc.compile()
        _CACHE["nc"] = nc
    return _CACHE["nc"]


def kernel(x, wq, bq, wk, bk, wv, bv, wo, bo):
    global LAST_RESULTS
    x = np.asarray(x, np.float32)
    wq, bq = np.asarray(wq, np.float32), np.asarray(bq, np.float32)
    wk, bk = np.asarray(wk, np.float32), np.asarray(bk, np.float32)
    wv, bv = np.asarray(wv, np.float32), np.asarray(bv, np.float32)
    wo, bo = np.asarray(wo, np.float32), np.asarray(bo, np.float32)
    B, TOK, D = x.shape          # (2, 2048, 1024)
    NH, DH = 4, 64               # heads per core, head dim
    DC = NH * DH                 # 256
    MQ = DC // P                 # 2
    KD = D // P                  # 8
    KC = KD // 2                 # 4
    BF = ml_dtypes.bfloat16
    E4 = ml_dtypes.float8_e4m3

    nc = _get_compiled()

    def chunk_rows(a, nchunk):
        # [R, C] -> [P, nchunk, C] with [p, c, :] = a[c*P + p, :]
        R, C = a.shape
        return np.ascontiguousarray(
            a.reshape(nchunk, P, C).transpose(1, 0, 2))

    def dr_chunks(a):
        # [R, C] -> [P, KC, 2, C] DoubleRow layout (row = c*256 + j*128 + p)
        c = chunk_rows(a, KD)  # [P, 8, C]
        return np.ascontiguousarray(
            c.reshape(P, KC, 2, a.shape[1]))

    def fp8_sets(a):
        # returns (a8, ar8, a8s): value + residual + 2^-5-scaled copies
        a8 = a.astype(E4)
        ar = (a - a8.astype(np.float32)).astype(E4)
        as_ = (a * 2.0 ** -5).astype(E4)
        return a8, ar, as_

    in_maps = []
    x8_b, xr_b, xs_b = [], [], []
    for b in range(B):
        x8, xr, xs = fp8_sets(x[b].T)  # [D, TOK]
        x8_b.append(dr_chunks(x8))
        xr_b.append(dr_chunks(xr))
        xs_b.append(dr_chunks(xs))
    for c in range(8):
        b, hg = c // 4, c % 4
        sl = slice(hg * DC, (hg + 1) * DC)
        m = {
            "x8": x8_b[b], "xr": xr_b[b], "xs": xs_b[b],
            "wo": chunk_rows(wo[sl, :], MQ).astype(BF),
            "bq": np.ascontiguousarray(bq[sl].reshape(MQ, P).T),
            "bk": np.ascontiguousarray(bk[sl].reshape(MQ, P).T),
            "bv": np.ascontiguousarray(np.tile(bv[None, sl], (P, 1))),
        }
        for nm, w in (("q", wq), ("k", wk), ("v", wv)):
            w8 = w[:, sl].astype(E4)
            wr = ((w[:, sl] - w8.astype(np.float32)) * 2.0 ** 5).astype(E4)
            m[f"w8{nm}"] = dr_chunks(w8)
            m[f"wr{nm}"] = dr_chunks(wr)
        in_maps.append(m)

    trace = os.environ.get("KERNEL_TRACE", "0") == "1"
    res = run_bass_kernel_spmd(nc, in_maps, core_ids=list(range(8)),
                               trace=trace)
    LAST_RESULTS = res
    outs = [res.results[c]["out"].astype(np.float32) for c in range(8)]
    y = np.stack([sum(outs[0:4]) + bo, sum(outs[4:8]) + bo], axis=0)
    return np.ascontiguousarray(y, dtype=np.float32)


# revision 63
# speedup vs baseline: 1.0024x; 1.0020x over previous
"""Multi-head self-attention TRN2 kernel (16 heads, D=1024, x:[2,2048,1024]).

Sharding: 8 cores = 2 (batch) x 4 (head groups of 4 heads). Host sums the
4 bf16 partials per batch (tensor-parallel all-reduce) and adds bo.

Per-core pipeline:
  QKV proj: error-compensated fp8e4m3 DoubleRow (half-rate PE):
        x@w ~= x8@w8 + xr8@w8 + x8s@wr8s  with host-prepared x8=e4m3(x),
        xr8=e4m3(x-x8), x8s=e4m3(x/32), wr8s=e4m3((w-w8)*32); kT/qT
        head-dim-major bf16 [256, 2048], v token-major [2048, 4, 65] bf16
        with a ones column (softmax sums fall out of the AV matmul).
  scores:  kT strips (K=64) x qT -> [128 keys, 512 q] f32 PSUM, bf16 ops.
  ex:      exp(s/8): ACT Exp (bf16 out) for 3/4 of (head, key-group)
           slices; DVE Schraudolph (affine+uint16 trunc = bf16 exp bits,
           mean-centered) for the rest -- engine-split is per whole
           key-group so softmax normalization cancels engine bias.
  o^T:     AV in q-major orientation: [128 q, 65] = ex_chunk^T @ v_chunk,
           4 q-subblocks packed per PSUM bank (one start=True zeroes the
           2KB bank; the rest accumulate onto pending-zero bytes).
  norm:    DVE reciprocal_approx_fast on the sums column + broadcast mult.
  oT:      XBAR DMA transpose [128 q, 128 dh-pair] -> oT_sb bf16.
  out:     oT^T @ wo (bf16) per 128-token chunk -> DVE bf16 copy -> DMA.
Projections for head-pair 1 / later q-blocks and the output projection
are streamed into the attention stages to hide the input DMA and keep
all engines busy.
"""

import os
import sys
from contextlib import ExitStack

import numpy as np

for _p in ("/opt/trn_rl_repo", os.path.expanduser("~/.axon_site/_ro/trn_rl_repo")):
    if os.path.isdir(_p) and _p not in sys.path:
        sys.path.insert(0, _p)

import ml_dtypes  # noqa: E402

import concourse.bass as bass  # noqa: E402
import concourse.mybir as mybir  # noqa: E402
import concourse.tile as tile  # noqa: E402
from concourse import bacc  # noqa: E402
from concourse.bass_utils import run_bass_kernel_spmd  # noqa: E402

f32 = mybir.dt.float32
bf16 = mybir.dt.bfloat16
u16 = mybir.dt.uint16
P = 128

# Schraudolph exp-as-bf16-bits: bits = A*s_raw + B, trunc to uint16.
# A = 128*log2(e)*0.125 (the 1/8 softmax scale folded in); B centers the
# piecewise-linear ripple (minimax) and compensates truncation.
SCH_A = 128.0 * float(np.log2(np.e)) * 0.125
SCH_B = 128.0 * 127.0 - 6.86


def build_core_program(D=1024, TOK=2048, NH=4, num_devices=8):
    """One core's program: 4 heads of one batch of the attention layer."""
    DH = 64
    KD = D // P          # hidden-dim 128-chunks (8)
    NQ = TOK // 512      # 512-wide q blocks (4)
    NT = TOK // P        # 128-wide token chunks (16)
    DC = NH * DH         # per-core head dims (256)
    MQ = DC // P         # 128-row chunks of qT/kT/oT (2)
    HPC = P // DH        # heads per 128-row chunk (2)
    OW = 512             # output column chunk width
    NO = D // OW         # output column chunks (2)
    G = NT // 2          # key-pair groups per stage (8)

    nc = bacc.Bacc("TRN2", target_bir_lowering=False, debug=False,
                   num_devices=num_devices)

    KC = KD // 2         # 256-row DoubleRow chunks (4)
    fp8 = mybir.dt.float8e4
    x8_d = nc.declare_dram_parameter("x8", [P, KC, 2, TOK], fp8, isOutput=False)
    xr_d = nc.declare_dram_parameter("xr", [P, KC, 2, TOK], fp8, isOutput=False)
    xs_d = nc.declare_dram_parameter("xs", [P, KC, 2, TOK], fp8, isOutput=False)
    w8_d = {}
    wr_d = {}
    for nm in ("q", "k", "v"):
        w8_d[nm] = nc.declare_dram_parameter(
            f"w8{nm}", [P, KC, 2, DC], fp8, isOutput=False)
        wr_d[nm] = nc.declare_dram_parameter(
            f"wr{nm}", [P, KC, 2, DC], fp8, isOutput=False)
    wo_d = nc.declare_dram_parameter("wo", [P, MQ, D], bf16, isOutput=False)
    bq_d = nc.declare_dram_parameter("bq", [P, MQ], f32, isOutput=False)
    bk_d = nc.declare_dram_parameter("bk", [P, MQ], f32, isOutput=False)
    bv_d = nc.declare_dram_parameter("bv", [P, DC], f32, isOutput=False)
    out_d = nc.declare_dram_parameter("out", [TOK, D], bf16, isOutput=True)

    with tile.TileContext(nc) as tc, ExitStack() as ctx:
        persist = ctx.enter_context(tc.tile_pool(name="persist", bufs=1))
        work = ctx.enter_context(tc.tile_pool(name="work", bufs=3))
        psp = ctx.enter_context(tc.tile_pool(name="psp", bufs=2, space="PSUM"))

        # ---- phase A: loads ------------------------------------------
        w8_sb = {nm: persist.tile([P, KC, 2, DC], fp8, name=f"w8{nm}")
                 for nm in ("q", "k", "v")}
        wr_sb = {nm: persist.tile([P, KC, 2, DC], fp8, name=f"wr{nm}")
                 for nm in ("q", "k", "v")}
        wo_sb = persist.tile([P, MQ, D], bf16)
        bq_sb = persist.tile([P, MQ], f32)
        bk_sb = persist.tile([P, MQ], f32)
        bv_sb = persist.tile([P, DC], f32)
        x8_sb = persist.tile([P, KC, 2, TOK], fp8)
        xr_sb = persist.tile([P, KC, 2, TOK], fp8)
        xs_sb = persist.tile([P, KC, 2, TOK], fp8)

        # startup-critical transfers first (kT0/qT0 2-set inputs), then the
        # stream in stage-(0,*) consumption order
        nc.gpsimd.dma_start(w8_sb["k"][:], w8_d["k"][:])
        nc.sync.dma_start(x8_sb[:, :, :, 0:256], x8_d[:, :, :, 0:256])
        nc.gpsimd.dma_start(w8_sb["q"][:], w8_d["q"][:])
        nc.sync.dma_start(xr_sb[:, :, :, 0:256], xr_d[:, :, :, 0:256])
        nc.gpsimd.dma_start(wr_sb["k"][:], wr_d["k"][:])
        nc.sync.dma_start(xs_sb[:, :, :, 0:256], xs_d[:, :, :, 0:256])
        nc.gpsimd.dma_start(wr_sb["q"][:], wr_d["q"][:])
        nc.sync.dma_start(x8_sb[:, :, :, 256:512], x8_d[:, :, :, 256:512])
        nc.gpsimd.dma_start(bk_sb[:], bk_d[:])
        nc.sync.dma_start(xr_sb[:, :, :, 256:512], xr_d[:, :, :, 256:512])
        nc.gpsimd.dma_start(bq_sb[:], bq_d[:])
        nc.sync.dma_start(xs_sb[:, :, :, 256:512], xs_d[:, :, :, 256:512])
        nc.gpsimd.dma_start(w8_sb["v"][:], w8_d["v"][:])
        nc.gpsimd.dma_start(wr_sb["v"][:], wr_d["v"][:])
        nc.gpsimd.dma_start(bv_sb[:], bv_d[:])
        for n in range(1, NQ):
            ns = slice(n * 512, (n + 1) * 512)
            for t_sb, t_d in ((x8_sb, x8_d), (xr_sb, xr_d), (xs_sb, xs_d)):
                nc.sync.dma_start(t_sb[:, :, :, ns], t_d[:, :, :, ns])
        nc.gpsimd.dma_start(wo_sb[:], wo_d[:])

        v_sb = persist.tile([P, NT, NH, DH + 1], bf16)
        nc.vector.memset(v_sb[:, :, :, DH:DH + 1], 1.0)

        qT_sb = persist.tile([P, MQ, TOK], bf16)
        kT_sb = persist.tile([P, MQ, TOK], bf16)
        oT_sb = persist.tile([P, MQ, TOK], bf16)

        # ---- phase B: kT/qT0 for head-pair 0 only; the rest of the
        # projections are interleaved into the first two stages --------
        DR = mybir.MatmulPerfMode.DoubleRow
        PROJ_SETS = (("8", x8_sb), ("8", xr_sb), ("r", xs_sb))

        def proj_block(nm, b_sb, t_sb, m, n, c0=0, c1=512):
            # t = (x8@w8 + xr8@w8 + x8s@wr8s) + b, fp8e4 DoubleRow
            ps = psp.tile([P, 512], f32, tag="acc", name="ps")
            ns = slice(n * 512 + c0, n * 512 + c1)
            wsb = {"8": w8_sb[nm], "r": wr_sb[nm]}
            for si, (wv_, xv) in enumerate(PROJ_SETS):
                for c in range(KC):
                    nc.tensor.matmul(
                        ps[:, c0:c1], wsb[wv_][:, c, :, m * P:(m + 1) * P],
                        xv[:, c, :, ns], perf_mode=DR,
                        start=(si == 0 and c == 0),
                        stop=(si == 2 and c == KC - 1))
            nc.vector.tensor_tensor(
                t_sb[:, m, ns], ps[:, c0:c1],
                b_sb[:, m:m + 1].to_broadcast([P, c1 - c0]),
                mybir.AluOpType.add)

        # half-block startup: projections and the first two score groups
        # consume 256-column halves so compute starts mid-DMA
        proj_block("k", bk_sb, kT_sb, 0, 0, 0, 256)
        proj_block("q", bq_sb, qT_sb, 0, 0, 0, 256)
        proj_block("k", bk_sb, kT_sb, 0, 0, 256, 512)
        proj_block("q", bq_sb, qT_sb, 0, 0, 256, 512)

        def emit_v_chunk(t, vp):
            ps = psp.tile([P, P], f32, tag="acc", name="vps")
            tsl = slice(t * P, (t + 1) * P)
            vsl = slice(vp * P, (vp + 1) * P)
            wsb = {"8": w8_sb["v"], "r": wr_sb["v"]}
            for si, (wv_, xv) in enumerate(PROJ_SETS):
                for c in range(KC):
                    nc.tensor.matmul(
                        ps[:], xv[:, c, :, tsl], wsb[wv_][:, c, :, vsl],
                        perf_mode=DR,
                        start=(si == 0 and c == 0),
                        stop=(si == 2 and c == KC - 1))
            nc.vector.tensor_tensor(
                v_sb[:, t, 2 * vp:2 * vp + 2, 0:DH],
                ps.rearrange("p (h d) -> p h d", h=HPC),
                bv_sb[:, vsl].rearrange("p (h d) -> p h d", h=HPC),
                mybir.AluOpType.add)

        # ---- phase C: attention + output projection ------------------
        def emit_oproj_tok(tok, fin=False):
            ts = slice(tok * P, (tok + 1) * P)
            for nn in range(NO):
                ns = slice(nn * OW, (nn + 1) * OW)
                op = psp.tile([P, OW], f32, tag="acc", name="op")
                for m in range(MQ):
                    nc.tensor.matmul(
                        op[:], oT_sb[:, m, ts], wo_sb[:, m, ns],
                        start=(m == 0), stop=(m == MQ - 1))
                ou = work.tile([P, OW], bf16, tag="ou", bufs=4, name="ou")
                nc.vector.tensor_copy(ou[:], op[:])
                (nc.sync if fin else nc.gpsimd).dma_start(
                    out_d[ts, ns], ou[:])

        def emit_oproj(n):
            for t in range(4):
                emit_oproj_tok(n * 4 + t)

        for n in range(NQ):
            qs = slice(n * 512, (n + 1) * 512)
            for hp in range(MQ):
                pair = [hp * HPC, hp * HPC + 1]
                avs = {}
                for h in pair:
                    avs[h] = psp.tile([P, 4, DH + 1], f32, tag="av",
                                      padded_shape=[P, 4, P], name=f"av{h}")

                def emit_av(g, exs, which):
                    for h in which:
                        ex = exs[h]
                        for j in range(2):
                            for q4 in range(4):
                                nc.tensor.matmul(
                                    avs[h][:, q4, :],
                                    ex[:, j, q4 * P:(q4 + 1) * P],
                                    v_sb[:, 2 * g + j, h, :],
                                    start=(g == 0 and j == 0 and q4 == 0),
                                    stop=(g == G - 1 and j == 1),
                                    skip_group_check=True)

                exs_hist = {}
                for g in range(G):
                    if n == 0 and g % 2 == 0 and g > 0:
                        # stream the kT block feeding this key-group pair
                        proj_block("k", bk_sb, kT_sb, hp, g // 2)
                    if n == 0 and hp == 0 and g == 1:
                        proj_block("k", bk_sb, kT_sb, 1, 0)

                    cur_exs = {}
                    for i, h in enumerate(pair):
                        sc = psp.tile([P, 2, 512], f32, tag="sc",
                                      name=f"sc{h}")
                        hm, hr = h // HPC, (h % HPC) * DH
                        halves = ((0, 512),) if not (n == 0 and hp == 0
                                                     and g < 2) \
                            else ((0, 256), (256, 512))
                        for j in range(2):
                            kk = g * 2 + j
                            for qa, qb in halves:
                                nc.tensor.matmul(
                                    sc[:, j, qa:qb],
                                    kT_sb[hr:hr + DH, hm,
                                          kk * P:(kk + 1) * P],
                                    qT_sb[hr:hr + DH, hm,
                                          n * 512 + qa:n * 512 + qb],
                                    start=True, stop=True,
                                    skip_group_check=True)
                        # exp: DVE Schraudolph for ~5/16 of (head, key-group)
                        # slices; ACT exact exp otherwise
                        if DVE_EXP and ((i == 1 and g % 2 == 0) or (i == 0 and g == 3)):
                            ex = work.tile([P, 2, 512], u16, bufs=6,
                                           tag=f"ex{i}", name="exu")
                            nc.vector.tensor_scalar(
                                ex[:], sc[:], SCH_A, SCH_B,
                                mybir.AluOpType.mult, mybir.AluOpType.add)
                            cur_exs[h] = ex.bitcast(bf16)
                        else:
                            ex = work.tile([P, 2, 512], bf16, bufs=6,
                                           tag=f"ex{i}", name="exb")
                            nc.scalar.activation(
                                ex[:], sc[:],
                                mybir.ActivationFunctionType.Exp, scale=0.125)
                            cur_exs[h] = ex
                        # AV lagged two groups behind scores/exp to absorb
                        # exp latency; halved between the heads' emissions
                        if g >= 2:
                            emit_av(g - 2, exs_hist[g - 2], [pair[i]])
                    if n == 0:
                        # v projections streamed just ahead of their AV use
                        emit_v_chunk(2 * g, hp)
                        emit_v_chunk(2 * g + 1, hp)
                        if hp == 0 and g == G - 1:
                            proj_block("q", bq_sb, qT_sb, 1, 0)
                    elif hp == 1 and g < 4:
                        # previous block's output projection, one token
                        # chunk per key-group to spread PSUM slot reuse
                        emit_oproj_tok((n - 1) * 4 + g, fin=(n == NQ - 1))
                    exs_hist[g] = cur_exs
                emit_av(G - 2, exs_hist[G - 2], pair)
                emit_av(G - 1, exs_hist[G - 1], pair)
                # normalize + transpose into oT (+ last-block oproj tail)
                last = (n == NQ - 1 and hp == MQ - 1)
                onorm = work.tile([P, 4, P], bf16, tag="onorm", bufs=4,
                                  name="onorm")
                for i, h in enumerate(pair):
                    rcp = work.tile([P, 4, 1], f32, tag="rcp", bufs=4,
                                    name="rcp")
                    nc.vector.reciprocal_approx_fast(
                        rcp[:], avs[h][:, :, DH:DH + 1])
                    nc.vector.tensor_tensor(
                        onorm[:, :, i * DH:(i + 1) * DH],
                        avs[h][:, :, 0:DH],
                        rcp.to_broadcast([P, 4, DH]),
                        mybir.AluOpType.mult)
                if n + 1 < NQ:
                    proj_block("q", bq_sb, qT_sb, hp, n + 1)
                for q4 in range(4):
                    nc.sync.dma_start_transpose(
                        oT_sb[:, hp, n * 512 + q4 * P: n * 512 + (q4 + 1) * P],
                        onorm[:, q4, :])
                    if last:
                        emit_oproj_tok(n * 4 + q4, fin=True)
    return nc


_CACHE = {}
LAST_RESULTS = None


def _get_compiled():
    if "nc" not in _CACHE:
        nc = build_core_program()
        nc.compile()
        _CACHE["nc"] = nc
    return _CACHE["nc"]


def kernel(x, wq, bq, wk, bk, wv, bv, wo, bo):
    global LAST_RESULTS
    x = np.asarray(x, np.float32)
    wq, bq = np.asarray(wq, np.float32), np.asarray(bq, np.float32)
    wk, bk = np.asarray(wk, np.float32), np.asarray(bk, np.float32)
    wv, bv = np.asarray(wv, np.float32), np.asarray(bv, np.float32)
    wo, bo = np.asarray(wo, np.float32), np.asarray(bo, np.float32)
    B, TOK, D = x.shape          # (2, 2048, 1024)
    NH, DH = 4, 64               # heads per core, head dim
    DC = NH * DH                 # 256
    MQ = DC // P                 # 2
    KD = D // P                  # 8
    KC = KD // 2                 # 4
    BF = ml_dtypes.bfloat16
    E4 = ml_dtypes.float8_e4m3

    nc = _get_compiled()

    def chunk_rows(a, nchunk):
        # [R, C] -> [P, nchunk, C] with [p, c, :] = a[c*P + p, :]
        R, C = a.shape
        return np.ascontiguousarray(
            a.reshape(nchunk, P, C).transpose(1, 0, 2))

    def dr_chunks(a):
        # [R, C] -> [P, KC, 2, C] DoubleRow layout (row = c*256 + j*128 + p)
        c = chunk_rows(a, KD)  # [P, 8, C]
        return np.ascontiguousarray(
            c.reshape(P, KC, 2, a.shape[1]))

    def fp8_sets(a):
        # returns (a8, ar8, a8s): value + residual + 2^-5-scaled copies
        a8 = a.astype(E4)
        ar = (a - a8.astype(np.float32)).astype(E4)
        as_ = (a * 2.0 ** -5).astype(E4)
        return a8, ar, as_

    in_maps = []
    x8_b, xr_b, xs_b = [], [], []
    for b in range(B):
        x8, xr, xs = fp8_sets(x[b].T)  # [D, TOK]
        x8_b.append(dr_chunks(x8))
        xr_b.append(dr_chunks(xr))
        xs_b.append(dr_chunks(xs))
    for c in range(8):
        b, hg = c // 4, c % 4
        sl = slice(hg * DC, (hg + 1) * DC)
        m = {
            "x8": x8_b[b], "xr": xr_b[b], "xs": xs_b[b],
            "wo": chunk_rows(wo[sl, :], MQ).astype(BF),
            "bq": np.ascontiguousarray(bq[sl].reshape(MQ, P).T),
            "bk": np.ascontiguousarray(bk[sl].reshape(MQ, P).T),
            "bv": np.ascontiguousarray(np.tile(bv[None, sl], (P, 1))),
        }
        for nm, w in (("q", wq), ("k", wk), ("v", wv)):
            w8 = w[:, sl].astype(E4)
            wr = ((w[:, sl] - w8.astype(np.float32)) * 2.0 ** 5).astype(E4)
            m[f"w8{nm}"] = dr_chunks(w8)
            m[f"wr{nm}"] = dr_chunks(wr)
        in_maps.append(m)

    trace = os.environ.get("KERNEL_TRACE", "0") == "1"
    res = run_bass_kernel_spmd(nc, in_maps, core_ids=list(range(8)),
                               trace=trace)
    LAST_RESULTS = res
    outs = [res.results[c]["out"].astype(np.float32) for c in range(8)]
    y = np.stack([sum(outs[0:4]) + bo, sum(outs[4:8]) + bo], axis=0)
    return np.ascontiguousarray(y, dtype=np.float32)


# revision 71
# speedup vs baseline: 1.0103x; 1.0078x over previous
"""Multi-head self-attention TRN2 kernel (16 heads, D=1024, x:[2,2048,1024]).

Sharding: 8 cores = 2 (batch) x 4 (head groups of 4 heads). Host sums the
4 bf16 partials per batch (tensor-parallel all-reduce) and adds bo.

Per-core pipeline:
  QKV proj: error-compensated fp8e4m3 DoubleRow (half-rate PE):
        x@w ~= x8@w8 + xr8@w8 + x8s@wr8s  with host-prepared x8=e4m3(x),
        xr8=e4m3(x-x8), x8s=e4m3(x/32), wr8s=e4m3((w-w8)*32); kT/qT
        head-dim-major bf16 [256, 2048], v token-major [2048, 4, 65] bf16
        with a ones column (softmax sums fall out of the AV matmul).
  scores:  kT strips (K=64) x qT -> [128 keys, 512 q] f32 PSUM, bf16 ops.
  ex:      exp(s/8): ACT Exp (bf16 out) for 3/4 of (head, key-group)
           slices; DVE Schraudolph (affine+uint16 trunc = bf16 exp bits,
           mean-centered) for the rest -- engine-split is per whole
           key-group so softmax normalization cancels engine bias.
  o^T:     AV in q-major orientation: [128 q, 65] = ex_chunk^T @ v_chunk,
           4 q-subblocks packed per PSUM bank (one start=True zeroes the
           2KB bank; the rest accumulate onto pending-zero bytes).
  norm:    DVE reciprocal_approx_fast on the sums column + broadcast mult.
  oT:      XBAR DMA transpose [128 q, 128 dh-pair] -> oT_sb bf16.
  out:     oT^T @ wo (bf16) per 128-token chunk -> DVE bf16 copy -> DMA.
Projections for head-pair 1 / later q-blocks and the output projection
are streamed into the attention stages to hide the input DMA and keep
all engines busy.
"""

import os
import sys
from contextlib import ExitStack

import numpy as np

for _p in ("/opt/trn_rl_repo", os.path.expanduser("~/.axon_site/_ro/trn_rl_repo")):
    if os.path.isdir(_p) and _p not in sys.path:
        sys.path.insert(0, _p)

import ml_dtypes  # noqa: E402

import concourse.bass as bass  # noqa: E402
import concourse.mybir as mybir  # noqa: E402
import concourse.tile as tile  # noqa: E402
from concourse import bacc  # noqa: E402
from concourse.bass_utils import run_bass_kernel_spmd  # noqa: E402

f32 = mybir.dt.float32
bf16 = mybir.dt.bfloat16
u16 = mybir.dt.uint16
P = 128

# Schraudolph exp-as-bf16-bits: bits = A*s_raw + B, trunc to uint16.
# A = 128*log2(e)*0.125 (the 1/8 softmax scale folded in); B centers the
# piecewise-linear ripple (minimax) and compensates truncation.
SCH_A = 128.0 * float(np.log2(np.e)) * 0.125
SCH_B = 128.0 * 127.0 - 6.86


def build_core_program(D=1024, TOK=2048, NH=4, num_devices=8):
    """One core's program: 4 heads of one batch of the attention layer."""
    DH = 64
    KD = D // P          # hidden-dim 128-chunks (8)
    NQ = TOK // 512      # 512-wide q blocks (4)
    NT = TOK // P        # 128-wide token chunks (16)
    DC = NH * DH         # per-core head dims (256)
    MQ = DC // P         # 128-row chunks of qT/kT/oT (2)
    HPC = P // DH        # heads per 128-row chunk (2)
    OW = 512             # output column chunk width
    NO = D // OW         # output column chunks (2)
    G = NT // 2          # key-pair groups per stage (8)

    nc = bacc.Bacc("TRN2", target_bir_lowering=False, debug=False,
                   num_devices=num_devices)

    KC = KD // 2         # 256-row DoubleRow chunks (4)
    fp8 = mybir.dt.float8e4
    x8_d = nc.declare_dram_parameter("x8", [P, KC, 2, TOK], fp8, isOutput=False)
    xr_d = nc.declare_dram_parameter("xr", [P, KC, 2, TOK], fp8, isOutput=False)
    xs_d = nc.declare_dram_parameter("xs", [P, KC, 2, TOK], fp8, isOutput=False)
    w8_d = {}
    wr_d = {}
    for nm in ("q", "k", "v"):
        w8_d[nm] = nc.declare_dram_parameter(
            f"w8{nm}", [P, KC, 2, DC], fp8, isOutput=False)
        wr_d[nm] = nc.declare_dram_parameter(
            f"wr{nm}", [P, KC, 2, DC], fp8, isOutput=False)
    wo_d = nc.declare_dram_parameter("wo", [P, MQ, D], bf16, isOutput=False)
    bq_d = nc.declare_dram_parameter("bq", [P, MQ], f32, isOutput=False)
    bk_d = nc.declare_dram_parameter("bk", [P, MQ], f32, isOutput=False)
    bv_d = nc.declare_dram_parameter("bv", [P, DC], f32, isOutput=False)
    out_d = nc.declare_dram_parameter("out", [TOK, D], bf16, isOutput=True)

    with tile.TileContext(nc) as tc, ExitStack() as ctx:
        persist = ctx.enter_context(tc.tile_pool(name="persist", bufs=1))
        work = ctx.enter_context(tc.tile_pool(name="work", bufs=3))
        psp = ctx.enter_context(tc.tile_pool(name="psp", bufs=2, space="PSUM"))

        # ---- phase A: loads ------------------------------------------
        w8_sb = {nm: persist.tile([P, KC, 2, DC], fp8, name=f"w8{nm}")
                 for nm in ("q", "k", "v")}
        wr_sb = {nm: persist.tile([P, KC, 2, DC], fp8, name=f"wr{nm}")
                 for nm in ("q", "k", "v")}
        wo_sb = persist.tile([P, MQ, D], bf16)
        bq_sb = persist.tile([P, MQ], f32)
        bk_sb = persist.tile([P, MQ], f32)
        bv_sb = persist.tile([P, DC], f32)
        x8_sb = persist.tile([P, KC, 2, TOK], fp8)
        xr_sb = persist.tile([P, KC, 2, TOK], fp8)
        xs_sb = persist.tile([P, KC, 2, TOK], fp8)

        # startup-critical transfers first (kT0/qT0 2-set inputs), then the
        # stream in stage-(0,*) consumption order
        nc.gpsimd.dma_start(w8_sb["k"][:], w8_d["k"][:])
        nc.sync.dma_start(x8_sb[:, :, :, 0:256], x8_d[:, :, :, 0:256])
        nc.gpsimd.dma_start(w8_sb["q"][:], w8_d["q"][:])
        nc.sync.dma_start(xr_sb[:, :, :, 0:256], xr_d[:, :, :, 0:256])
        nc.gpsimd.dma_start(wr_sb["k"][:], wr_d["k"][:])
        nc.sync.dma_start(xs_sb[:, :, :, 0:256], xs_d[:, :, :, 0:256])
        nc.gpsimd.dma_start(wr_sb["q"][:], wr_d["q"][:])
        nc.sync.dma_start(x8_sb[:, :, :, 256:512], x8_d[:, :, :, 256:512])
        nc.gpsimd.dma_start(bk_sb[:], bk_d[:])
        nc.sync.dma_start(xr_sb[:, :, :, 256:512], xr_d[:, :, :, 256:512])
        nc.gpsimd.dma_start(bq_sb[:], bq_d[:])
        nc.sync.dma_start(xs_sb[:, :, :, 256:512], xs_d[:, :, :, 256:512])
        nc.gpsimd.dma_start(w8_sb["v"][:], w8_d["v"][:])
        nc.gpsimd.dma_start(wr_sb["v"][:], wr_d["v"][:])
        nc.gpsimd.dma_start(bv_sb[:], bv_d[:])
        for n in range(1, NQ):
            ns = slice(n * 512, (n + 1) * 512)
            for ti, (t_sb, t_d) in enumerate(
                    ((x8_sb, x8_d), (xr_sb, xr_d), (xs_sb, xs_d))):
                eng = nc.sync if ti % 2 == 0 else nc.scalar
                eng.dma_start(t_sb[:, :, :, ns], t_d[:, :, :, ns])
        nc.gpsimd.dma_start(wo_sb[:], wo_d[:])

        v_sb = persist.tile([P, NT, NH, DH + 1], bf16)
        nc.vector.memset(v_sb[:, :, :, DH:DH + 1], 1.0)

        qT_sb = persist.tile([P, MQ, TOK], bf16)
        kT_sb = persist.tile([P, MQ, TOK], bf16)
        oT_sb = persist.tile([P, MQ, TOK], bf16)

        # ---- phase B: kT/qT0 for head-pair 0 only; the rest of the
        # projections are interleaved into the first two stages --------
        DR = mybir.MatmulPerfMode.DoubleRow
        PROJ_SETS = (("8", x8_sb), ("8", xr_sb), ("r", xs_sb))

        def proj_block(nm, b_sb, t_sb, m, n, c0=0, c1=512):
            # t = (x8@w8 + xr8@w8 + x8s@wr8s) + b, fp8e4 DoubleRow
            ps = psp.tile([P, 512], f32, tag="acc", name="ps")
            ns = slice(n * 512 + c0, n * 512 + c1)
            wsb = {"8": w8_sb[nm], "r": wr_sb[nm]}
            for si, (wv_, xv) in enumerate(PROJ_SETS):
                for c in range(KC):
                    nc.tensor.matmul(
                        ps[:, c0:c1], wsb[wv_][:, c, :, m * P:(m + 1) * P],
                        xv[:, c, :, ns], perf_mode=DR,
                        start=(si == 0 and c == 0),
                        stop=(si == 2 and c == KC - 1))
            nc.vector.tensor_tensor(
                t_sb[:, m, ns], ps[:, c0:c1],
                b_sb[:, m:m + 1].to_broadcast([P, c1 - c0]),
                mybir.AluOpType.add)

        # half-block startup: projections and the first two score groups
        # consume 256-column halves so compute starts mid-DMA
        proj_block("k", bk_sb, kT_sb, 0, 0, 0, 256)
        proj_block("q", bq_sb, qT_sb, 0, 0, 0, 256)
        proj_block("k", bk_sb, kT_sb, 0, 0, 256, 512)
        proj_block("q", bq_sb, qT_sb, 0, 0, 256, 512)

        def emit_v_chunk(t, vp):
            ps = psp.tile([P, P], f32, tag="acc", name="vps")
            tsl = slice(t * P, (t + 1) * P)
            vsl = slice(vp * P, (vp + 1) * P)
            wsb = {"8": w8_sb["v"], "r": wr_sb["v"]}
            for si, (wv_, xv) in enumerate(PROJ_SETS):
                for c in range(KC):
                    nc.tensor.matmul(
                        ps[:], xv[:, c, :, tsl], wsb[wv_][:, c, :, vsl],
                        perf_mode=DR,
                        start=(si == 0 and c == 0),
                        stop=(si == 2 and c == KC - 1))
            nc.vector.tensor_tensor(
                v_sb[:, t, 2 * vp:2 * vp + 2, 0:DH],
                ps.rearrange("p (h d) -> p h d", h=HPC),
                bv_sb[:, vsl].rearrange("p (h d) -> p h d", h=HPC),
                mybir.AluOpType.add)

        # ---- phase C: attention + output projection ------------------
        def emit_oproj_tok(tok, fin=False):
            ts = slice(tok * P, (tok + 1) * P)
            for nn in range(NO):
                ns = slice(nn * OW, (nn + 1) * OW)
                op = psp.tile([P, OW], f32, tag="acc", name="op")
                for m in range(MQ):
                    nc.tensor.matmul(
                        op[:], oT_sb[:, m, ts], wo_sb[:, m, ns],
                        start=(m == 0), stop=(m == MQ - 1))
                ou = work.tile([P, OW], bf16, tag="ou", bufs=4, name="ou")
                nc.vector.tensor_copy(ou[:], op[:])
                (nc.sync if fin else nc.gpsimd).dma_start(
                    out_d[ts, ns], ou[:])

        def emit_oproj(n):
            for t in range(4):
                emit_oproj_tok(n * 4 + t)

        for n in range(NQ):
            qs = slice(n * 512, (n + 1) * 512)
            for hp in range(MQ):
                pair = [hp * HPC, hp * HPC + 1]
                avs = {}
                for h in pair:
                    avs[h] = psp.tile([P, 4, DH + 1], f32, tag="av",
                                      padded_shape=[P, 4, P], name=f"av{h}")

                def emit_av(g, exs, which):
                    for h in which:
                        ex = exs[h]
                        for j in range(2):
                            for q4 in range(4):
                                nc.tensor.matmul(
                                    avs[h][:, q4, :],
                                    ex[:, j, q4 * P:(q4 + 1) * P],
                                    v_sb[:, 2 * g + j, h, :],
                                    start=(g == 0 and j == 0 and q4 == 0),
                                    stop=(g == G - 1 and j == 1),
                                    skip_group_check=True)

                exs_hist = {}
                for g in range(G):
                    if n == 0 and g % 2 == 0 and g > 0:
                        # stream the kT block feeding this key-group pair
                        proj_block("k", bk_sb, kT_sb, hp, g // 2)
                    if n == 0 and hp == 0 and g == 1:
                        proj_block("k", bk_sb, kT_sb, 1, 0)

                    cur_exs = {}
                    for i, h in enumerate(pair):
                        sc = psp.tile([P, 2, 512], f32, tag="sc",
                                      name=f"sc{h}")
                        hm, hr = h // HPC, (h % HPC) * DH
                        halves = ((0, 512),) if not (n == 0 and hp == 0
                                                     and g < 2) \
                            else ((0, 256), (256, 512))
                        for j in range(2):
                            kk = g * 2 + j
                            for qa, qb in halves:
                                nc.tensor.matmul(
                                    sc[:, j, qa:qb],
                                    kT_sb[hr:hr + DH, hm,
                                          kk * P:(kk + 1) * P],
                                    qT_sb[hr:hr + DH, hm,
                                          n * 512 + qa:n * 512 + qb],
                                    start=True, stop=True,
                                    skip_group_check=True)
                        # exp: DVE Schraudolph for ~5/16 of (head, key-group)
                        # slices; ACT exact exp otherwise
                        if DVE_EXP and ((i == 1 and g % 2 == 0) or (i == 0 and g == 3)):
                            ex = work.tile([P, 2, 512], u16, bufs=6,
                                           tag=f"ex{i}", name="exu")
                            nc.vector.tensor_scalar(
                                ex[:], sc[:], SCH_A, SCH_B,
                                mybir.AluOpType.mult, mybir.AluOpType.add)
                            cur_exs[h] = ex.bitcast(bf16)
                        else:
                            ex = work.tile([P, 2, 512], bf16, bufs=6,
                                           tag=f"ex{i}", name="exb")
                            nc.scalar.activation(
                                ex[:], sc[:],
                                mybir.ActivationFunctionType.Exp, scale=0.125)
                            cur_exs[h] = ex
                        # AV lagged two groups behind scores/exp to absorb
                        # exp latency; halved between the heads' emissions
                        if g >= 2:
                            emit_av(g - 2, exs_hist[g - 2], [pair[i]])
                    if n == 0:
                        # v projections streamed just ahead of their AV use
                        emit_v_chunk(2 * g, hp)
                        emit_v_chunk(2 * g + 1, hp)
                        if hp == 0 and g == G - 1:
                            proj_block("q", bq_sb, qT_sb, 1, 0)
                    elif hp == 1 and g < 4:
                        # previous block's output projection, one token
                        # chunk per key-group to spread PSUM slot reuse
                        emit_oproj_tok((n - 1) * 4 + g, fin=(n == NQ - 1))
                    exs_hist[g] = cur_exs
                emit_av(G - 2, exs_hist[G - 2], pair)
                emit_av(G - 1, exs_hist[G - 1], pair)
                # normalize + transpose into oT (+ last-block oproj tail)
                last = (n == NQ - 1 and hp == MQ - 1)
                onorm = work.tile([P, 4, P], bf16, tag="onorm", bufs=4,
                                  name="onorm")
                for i, h in enumerate(pair):
                    rcp = work.tile([P, 4, 1], f32, tag="rcp", bufs=4,
                                    name="rcp")
                    nc.vector.reciprocal_approx_fast(
                        rcp[:], avs[h][:, :, DH:DH + 1])
                    nc.vector.tensor_tensor(
                        onorm[:, :, i * DH:(i + 1) * DH],
                        avs[h][:, :, 0:DH],
                        rcp.to_broadcast([P, 4, DH]),
                        mybir.AluOpType.mult)
                if n + 1 < NQ:
                    proj_block("q", bq_sb, qT_sb, hp, n + 1)
                for q4 in range(4):
                    nc.sync.dma_start_transpose(
                        oT_sb[:, hp, n * 512 + q4 * P: n * 512 + (q4 + 1) * P],
                        onorm[:, q4, :])
                    if last:
                        emit_oproj_tok(n * 4 + q4, fin=True)
    return nc


_CACHE = {}
LAST_RESULTS = None


def _get_compiled():
    if "nc" not in _CACHE:
        nc = build_core_program()
        nc.compile()
        _CACHE["nc"] = nc
    return _CACHE["nc"]


def kernel(x, wq, bq, wk, bk, wv, bv, wo, bo):
    global LAST_RESULTS
    x = np.asarray(x, np.float32)
    wq, bq = np.asarray(wq, np.float32), np.asarray(bq, np.float32)
    wk, bk = np.asarray(wk, np.float32), np.asarray(bk, np.float32)
    wv, bv = np.asarray(wv, np.float32), np.asarray(bv, np.float32)
    wo, bo = np.asarray(wo, np.float32), np.asarray(bo, np.float32)
    B, TOK, D = x.shape          # (2, 2048, 1024)
    NH, DH = 4, 64               # heads per core, head dim
    DC = NH * DH                 # 256
    MQ = DC // P                 # 2
    KD = D // P                  # 8
    KC = KD // 2                 # 4
    BF = ml_dtypes.bfloat16
    E4 = ml_dtypes.float8_e4m3

    nc = _get_compiled()

    def chunk_rows(a, nchunk):
        # [R, C] -> [P, nchunk, C] with [p, c, :] = a[c*P + p, :]
        R, C = a.shape
        return np.ascontiguousarray(
            a.reshape(nchunk, P, C).transpose(1, 0, 2))

    def dr_chunks(a):
        # [R, C] -> [P, KC, 2, C] DoubleRow layout (row = c*256 + j*128 + p)
        c = chunk_rows(a, KD)  # [P, 8, C]
        return np.ascontiguousarray(
            c.reshape(P, KC, 2, a.shape[1]))

    def fp8_sets(a):
        # returns (a8, ar8, a8s): value + residual + 2^-5-scaled copies
        a8 = a.astype(E4)
        ar = (a - a8.astype(np.float32)).astype(E4)
        as_ = (a * 2.0 ** -5).astype(E4)
        return a8, ar, as_

    in_maps = []
    x8_b, xr_b, xs_b = [], [], []
    for b in range(B):
        x8, xr, xs = fp8_sets(x[b].T)  # [D, TOK]
        x8_b.append(dr_chunks(x8))
        xr_b.append(dr_chunks(xr))
        xs_b.append(dr_chunks(xs))
    for c in range(8):
        b, hg = c // 4, c % 4
        sl = slice(hg * DC, (hg + 1) * DC)
        m = {
            "x8": x8_b[b], "xr": xr_b[b], "xs": xs_b[b],
            "wo": chunk_rows(wo[sl, :], MQ).astype(BF),
            "bq": np.ascontiguousarray(bq[sl].reshape(MQ, P).T),
            "bk": np.ascontiguousarray(bk[sl].reshape(MQ, P).T),
            "bv": np.ascontiguousarray(np.tile(bv[None, sl], (P, 1))),
        }
        for nm, w in (("q", wq), ("k", wk), ("v", wv)):
            w8 = w[:, sl].astype(E4)
            wr = ((w[:, sl] - w8.astype(np.float32)) * 2.0 ** 5).astype(E4)
            m[f"w8{nm}"] = dr_chunks(w8)
            m[f"wr{nm}"] = dr_chunks(wr)
        in_maps.append(m)

    trace = os.environ.get("KERNEL_TRACE", "0") == "1"
    res = run_bass_kernel_spmd(nc, in_maps, core_ids=list(range(8)),
                               trace=trace)
    LAST_RESULTS = res
    outs = [res.results[c]["out"].astype(np.float32) for c in range(8)]
    y = np.stack([sum(outs[0:4]) + bo, sum(outs[4:8]) + bo], axis=0)
    return np.ascontiguousarray(y, dtype=np.float32)
